# revision 12
# baseline (speedup 1.0000x reference)
"""Multi-head attention (B=4, S=2048, D=1024, H=16, causal) on 8 TRN2 NeuronCores.

Sharding: 2 cores per batch element x 8 heads per core (data parallel over
batch, tensor parallel over heads). Each core computes, for its (batch, 8
heads): the QKV projection, causal softmax attention, and its partial
contribution to the output projection. The host pre-transposes x (so all
device matmuls see contraction dims on SBUF partitions), sums the two
per-batch partial projections, adds b_proj, and re-lays-out k/v into the
`present` tensor.

All matmuls run in float32r (TF32-like: fp32 with mantissa rounded to 11
bits by the PE) -- full PE throughput with ~1.5e-4 relative error.
"""

import sys
import numpy as np


def _ensure_paths():
    for p in ("/opt/trn_rl_repo", "/root/.axon_site/_ro/trn_rl_repo"):
        if p not in sys.path:
            sys.path.append(p)
    try:
        import concourse  # noqa: F401
    except ImportError:
        raise RuntimeError("concourse (bass) not importable")


_ensure_paths()

import concourse.bass as bass  # noqa: E402
import concourse.tile as tile  # noqa: E402
import concourse.mybir as mybir  # noqa: E402
from concourse import bacc  # noqa: E402
from concourse.bass import ts  # noqa: E402

B, S, D, H, KD = 4, 2048, 1024, 16, 64
HPC = H // 2          # 8 heads per core
N_CORES = 8
SQT = 512             # sq tile width (matmul moving-operand max for 4-byte)
NT = S // SQT         # 4 sq tiles
NCH = S // 128        # 16 sk chunks
DC = D // 128         # 8 contraction chunks for the QKV projection
PC = HPC * KD         # 512 = per-core projection contraction (8 heads x 64)
F32 = mybir.dt.float32
F32R = mybir.dt.float32r
EXP_SCALE = 1.0 / np.sqrt(np.float32(KD))  # folded into the exp activation

_prog_cache = {}


def _build_program(phases="ABC"):
    if phases in _prog_cache:
        return _prog_cache[phases]
    from contextlib import ExitStack

    nc = bacc.Bacc("TRN2", target_bir_lowering=False, debug=False,
                   num_devices=N_CORES)

    xt_d = nc.dram_tensor("xt", [D, S], F32R, kind="ExternalInput").ap()
    wqk_d = nc.dram_tensor("wqk", [D, 2 * PC], F32R, kind="ExternalInput").ap()
    wv_d = nc.dram_tensor("wv", [D, PC], F32R, kind="ExternalInput").ap()
    wp_d = nc.dram_tensor("wp", [PC, D], F32R, kind="ExternalInput").ap()
    bqk_d = nc.dram_tensor("bqk", [1, 2 * PC], F32R, kind="ExternalInput").ap()
    bv_d = nc.dram_tensor("bv", [1, PC], F32R, kind="ExternalInput").ap()
    tri_d = nc.dram_tensor("tri", [128, 128], F32R, kind="ExternalInput").ap()

    outp_d = nc.dram_tensor("outp", [S, D], F32, kind="ExternalOutput").ap()
    kt_d = nc.dram_tensor("kt", [PC, S], F32, kind="ExternalOutput").ap()
    vn_d = nc.dram_tensor("vn", [S, PC], F32, kind="ExternalOutput").ap()

    NQK = 2 * PC // 128   # 8 n-tiles: 0-3 are q^T row blocks, 4-7 are k^T

    class _SkipRest(Exception):
        pass

    def _body(tc, ctx):
        # ---- long-lived pools -------------------------------------------
        misc = ctx.enter_context(tc.tile_pool(name="misc", bufs=1))
        qkt_pool = ctx.enter_context(tc.tile_pool(name="qkt", bufs=1))
        vaug_pool = ctx.enter_context(tc.tile_pool(name="vaug", bufs=1))
        stage_pool = ctx.enter_context(tc.tile_pool(name="stage", bufs=3))

        tri = misc.tile([128, 128], F32R, name="tri")
        nc.sync.dma_start(tri[:], tri_d[:])
        ones = misc.tile([1, SQT], F32R, name="ones")
        nc.vector.memset(ones[:].bitcast(F32), 1.0)
        bqk = misc.tile([1, 2 * PC], F32R, name="bqk")
        nc.sync.dma_start(bqk[:], bqk_d[:])
        bv = misc.tile([1, PC], F32R, name="bv")
        nc.sync.dma_start(bv[:], bv_d[:])

        qkt = [qkt_pool.tile([128, S], F32R, name=f"qkt{n}") for n in range(NQK)]
        vaug = vaug_pool.tile([128, NCH, HPC * 65], F32R, name="vaug")
        for h in range(HPC):
            nc.vector.memset(vaug[:, :, h * 65 + 64 : h * 65 + 65].bitcast(F32), 1.0)

        # ---- phase A: QKV projection ------------------------------------
        with tc.tile_pool(name="wqk", bufs=1) as wqk_pool, \
             tc.tile_pool(name="wv", bufs=1) as wv_pool, \
             tc.tile_pool(name="xts", bufs=2) as xts_pool, \
             tc.tile_pool(name="psA", bufs=2, space="PSUM") as psA, \
             tc.tile_pool(name="psV", bufs=2, space="PSUM") as psV:

            wqk = []
            for c in range(DC):
                w = wqk_pool.tile([128, 2 * PC], F32R, name=f"wqk{c}")
                nc.sync.dma_start(w[:], wqk_d[ts(c, 128), :])
                wqk.append(w)
            wv = []
            for c in range(DC):
                w = wv_pool.tile([128, PC], F32R, name=f"wv{c}")
                nc.sync.dma_start(w[:], wv_d[ts(c, 128), :])
                wv.append(w)

            xt_r = xt_d.rearrange("(c p) s -> p c s", p=128)
            for t in range(NT):
                xts = xts_pool.tile([128, DC, SQT], F32R, name="xts")
                nc.sync.dma_start(xts[:], xt_r[:, :, ts(t, SQT)])

                for n in range(NQK):
                    ps = psA.tile([128, SQT], F32, name="psqk")
                    for c in range(DC):
                        nc.tensor.matmul(ps[:], wqk[c][:, ts(n, 128)],
                                         xts[:, c, :], start=(c == 0), stop=False)
                    nc.tensor.matmul(ps[:], bqk[:, ts(n, 128)],
                                     ones[:], start=False, stop=True)
                    nc.vector.tensor_copy(qkt[n][:, ts(t, SQT)], ps[:])
                    if n >= 4:  # also emit full-precision k for `present`
                        st = stage_pool.tile([128, SQT], F32, name="kst")
                        nc.scalar.copy(st[:], ps[:])
                        nc.sync.dma_start(kt_d[ts(n - 4, 128), ts(t, SQT)], st[:])

                for m in range(4):
                    cg = 4 * t + m  # global sk chunk
                    ps = psV.tile([128, PC], F32, name="psv")
                    for c in range(DC):
                        nc.tensor.matmul(ps[:], xts[:, c, ts(m, 128)],
                                         wv[c][:], start=(c == 0), stop=False)
                    nc.tensor.matmul(ps[:], ones[:, 0:128], bv[:],
                                     start=False, stop=True)
                    nc.vector.tensor_copy(
                        vaug[:, cg, :].rearrange("p (h e) -> p h e", e=65)[:, :, 0:64],
                        ps.rearrange("p (h d) -> p h d", d=KD))
                    st = stage_pool.tile([128, PC], F32, name="vst")
                    nc.scalar.copy(st[:], ps[:])
                    nc.sync.dma_start(vn_d[ts(cg, 128), :], st[:])

        # ---- phase B: attention + phase C: output projection -------------
        if "B" not in phases:
            raise _SkipRest()
        with tc.tile_pool(name="attn", bufs=1) as attn_pool, \
             tc.tile_pool(name="expp", bufs=3) as exp_pool, \
             tc.tile_pool(name="wp", bufs=1) as wp_pool, \
             tc.tile_pool(name="outst", bufs=2) as out_pool, \
             tc.tile_pool(name="rcp", bufs=2) as rcp_pool, \
             tc.tile_pool(name="psS", bufs=2, space="PSUM") as psS, \
             tc.tile_pool(name="psAV", bufs=2, space="PSUM") as psAV, \
             tc.tile_pool(name="psC", bufs=2, space="PSUM") as psC:

            wp = []
            for c in range(PC // 128):
                w = wp_pool.tile([128, D], F32R, name=f"wp{c}")
                nc.sync.dma_start(w[:], wp_d[ts(c, 128), :])
                wp.append(w)
            attn = [attn_pool.tile([128, S], F32R, name=f"attn{n}")
                    for n in range(PC // 128)]

            for t in range(NT):
                for h in range(HPC):
                    hp, ho = h // 2, (h % 2) * 64
                    qt = qkt[hp][ho:ho + 64, ts(t, SQT)]
                    ktile = qkt[4 + hp]
                    ps_av = psAV.tile([128, SQT], F32, name="psav")
                    nchunks = 4 * t + 4
                    for g in range((nchunks + 1) // 2):
                        pss = psS.tile([128, 2 * SQT], F32, name="pss")
                        ex = exp_pool.tile([128, 2 * SQT], F32R, name="ex")
                        cpair = [c for c in (2 * g, 2 * g + 1) if c < nchunks]
                        full = [c for c in cpair if c < 4 * t]
                        part = [c for c in cpair if c >= 4 * t]
                        for i, c in enumerate(cpair):
                            o = 0 if c < 4 * t else (c - 4 * t) * 128
                            nc.tensor.matmul(
                                pss[:, i * SQT + o : (i + 1) * SQT],
                                ktile[ho:ho + 64, ts(c, 128)],
                                qt[:, o:SQT] if o else qt,
                                start=True, stop=True)
                        if len(full) == 2:
                            nc.scalar.activation(ex[:], pss[:],
                                                 mybir.ActivationFunctionType.Exp,
                                                 scale=float(EXP_SCALE))
                        else:
                            for i, c in enumerate(cpair):
                                o = 0 if c < 4 * t else (c - 4 * t) * 128
                                sl = slice(i * SQT + o, (i + 1) * SQT)
                                nc.scalar.activation(ex[:, sl], pss[:, sl],
                                                     mybir.ActivationFunctionType.Exp,
                                                     scale=float(EXP_SCALE))
                        for i, c in enumerate(cpair):
                            if c in part:
                                o = (c - 4 * t) * 128
                                dsl = slice(i * SQT + o, i * SQT + o + 128)
                                nc.vector.tensor_mul(ex[:, dsl], ex[:, dsl], tri[:])
                        for i, c in enumerate(cpair):
                            o = 0 if c < 4 * t else (c - 4 * t) * 128
                            nc.tensor.matmul(
                                ps_av[0:65, o:SQT],
                                vaug[:, c, h * 65 : h * 65 + 65],
                                ex[:, i * SQT + o : (i + 1) * SQT],
                                start=(c == 0), stop=(c == nchunks - 1))
                    rc = rcp_pool.tile([1, SQT], F32, name="rc")
                    nc.vector.reciprocal(rc[:], ps_av[64:65, :])
                    rb = rcp_pool.tile([64, SQT], F32, name="rb")
                    nc.gpsimd.partition_broadcast(rb[:], rc[:])
                    nc.vector.tensor_mul(attn[hp][ho:ho + 64, ts(t, SQT)],
                                         ps_av[0:64, :], rb[:])

                for mm in range(4):
                    if "C" not in phases:
                        break
                    ost = out_pool.tile([128, D], F32, name="ost")
                    for nn in range(2):
                        ps = psC.tile([128, SQT], F32, name="psc")
                        for c in range(PC // 128):
                            nc.tensor.matmul(ps[:],
                                             attn[c][:, t * SQT + mm * 128:
                                                     t * SQT + (mm + 1) * 128],
                                             wp[c][:, ts(nn, SQT)],
                                             start=(c == 0), stop=(c == 3))
                        nc.vector.tensor_copy(ost[:, ts(nn, SQT)], ps[:])
                    nc.sync.dma_start(outp_d[ts(4 * t + mm, 128), :], ost[:])

    with tile.TileContext(nc) as tc, ExitStack() as ctx:
        try:
            _body(tc, ctx)
        except _SkipRest:
            pass

    nc.compile()
    _prog_cache[phases] = nc
    return nc


def _make_in_maps(x, W_attn, b_attn, W_proj, b_proj):
    x = np.asarray(x, dtype=np.float32)
    W_attn = np.asarray(W_attn, dtype=np.float32)
    b_attn = np.asarray(b_attn, dtype=np.float32)
    W_proj = np.asarray(W_proj, dtype=np.float32)

    # reference column order of the fused projection: value | query | key
    Wv_all = W_attn[:, 0:D]
    Wq_all = W_attn[:, D:2 * D]
    Wk_all = W_attn[:, 2 * D:3 * D]
    bv_all, bq_all, bk_all = b_attn[0:D], b_attn[D:2 * D], b_attn[2 * D:3 * D]

    tri = (np.arange(128)[:, None] <= np.arange(128)[None, :]).astype(np.float32)

    in_maps = []
    for core in range(N_CORES):
        b, half = divmod(core, 2)
        hs = half * HPC
        sl = slice(hs * KD, (hs + HPC) * KD)
        wqk = np.concatenate([Wq_all[:, sl], Wk_all[:, sl]], axis=1)
        bqk = np.concatenate([bq_all[sl], bk_all[sl]])[None, :]
        in_maps.append({
            "xt": np.ascontiguousarray(x[b].T),
            "wqk": np.ascontiguousarray(wqk),
            "wv": np.ascontiguousarray(Wv_all[:, sl]),
            "wp": np.ascontiguousarray(W_proj[sl, :]),
            "bqk": np.ascontiguousarray(bqk),
            "bv": np.ascontiguousarray(bv_all[sl][None, :]),
            "tri": tri,
        })
    return in_maps


def _gather(results, b_proj):
    b_proj = np.asarray(b_proj, dtype=np.float32)
    out = np.empty((B, S, D), dtype=np.float32)
    present = np.empty((2, B, H, S, KD), dtype=np.float32)
    for core in range(N_CORES):
        b, half = divmod(core, 2)
        hs = half * HPC
        r = results[core]
        if half == 0:
            out[b] = r["outp"]
        else:
            out[b] += r["outp"]
            out[b] += b_proj
        present[0, b, hs:hs + HPC] = r["kt"].reshape(HPC, KD, S).transpose(0, 2, 1)
        present[1, b, hs:hs + HPC] = r["vn"].reshape(S, HPC, KD).transpose(1, 0, 2)
    return out, present


def kernel(x, W_attn, b_attn, W_proj, b_proj, _trace=False, _trace_kwargs=None):
    from concourse.bass_utils import run_bass_kernel_spmd

    nc = _build_program()
    in_maps = _make_in_maps(x, W_attn, b_attn, W_proj, b_proj)
    res = run_bass_kernel_spmd(nc, in_maps, list(range(N_CORES)),
                               trace=_trace, **(_trace_kwargs or {}))
    out, present = _gather(res.results, b_proj)
    if _trace:
        kernel.last_result = res
    return out, present
